# revision 13
# baseline (speedup 1.0000x reference)
"""Trainium2 Bass kernel for nn_ConditionInjection (GroupNorm + rank-2-conditioned
cross-attention + output projection + residual).

Math notes (validated against the fp32 jax reference, absmax err ~2e-6):

  - q comes from only DC=2 condition channels, so the QK^T logits are rank-3:
      logits[i,j] * 1 = scale^2 * (qori[i,0]*kq[j,0] + qori[i,1]*kq[j,1] + kb[j])
    with [kq | kb] = h2 @ (fp1_wk.T @ [fp2_w | fp2_b]).  This replaces the
    K=256 contraction with K=3.
  - The output projection folds into V:  vw = h2 @ (fp1_wv.T @ out_w.T); the
    constant biases (out_w @ fp1_bv + out_b) bypass softmax (rows sum to 1)
    and become a final per-channel bias.
  - K-side biases shift logits uniformly per query and cancel in softmax.
  - max |logit| ~ 0.12, so exp() without max-subtraction is safe.
  - GroupNorm is scale-invariant: feeding x/sqrt(2) (the residual scaling)
    with eps/2 gives exactly the reference h2.

Sharding: data-parallel over the batch dim, B=32 -> 4 samples per core x 8.
"""

import os
import numpy as np
from contextlib import ExitStack

import concourse.bass as bass
import concourse.tile as tile
from concourse import bacc, mybir
from concourse import bass_utils

N_CORES = 8
B, C, H, W = 32, 256, 32, 32
S = H * W                      # 1024 spatial positions
BP = B // N_CORES              # samples per core
DC = 2
GROUPS = 32
CPG = C // GROUPS              # channels per group
EPS = 1e-5
R2 = float(1.0 / np.sqrt(2.0))
F32 = mybir.dt.float32
BF16 = mybir.dt.bfloat16

# Stash of the last run's results (test.py reads exec_time_ns from here).
LAST_RESULTS = None

_PROGRAM_CACHE = {}


def _build_program():
    nc = bacc.Bacc("TRN2", debug=False, num_devices=N_CORES)

    x_d = nc.dram_tensor("x", [BP, C, S], F32, kind="ExternalInput").ap()
    cm_d = nc.dram_tensor("cond", [BP, DC, 128, 128], F32, kind="ExternalInput").ap()
    wvt_d = nc.dram_tensor("wvt", [C, C], F32, kind="ExternalInput").ap()
    wk3_d = nc.dram_tensor("wk3", [C, 3], F32, kind="ExternalInput").ap()
    # aux columns: 0:2 gn_w halves, 2:4 gn_b halves, 4:6 final bias halves
    aux_d = nc.dram_tensor("aux", [128, 6], F32, kind="ExternalInput").ap()
    g1_d = nc.dram_tensor("g1", [128, GROUPS // 2], F32, kind="ExternalInput").ap()
    g2_d = nc.dram_tensor("g2", [GROUPS // 2, 128], F32, kind="ExternalInput").ap()
    out_d = nc.dram_tensor("out", [BP, C, S], F32, kind="ExternalOutput").ap()

    with tile.TileContext(nc) as tc, ExitStack() as ctx:
        wpool = ctx.enter_context(tc.tile_pool(name="weights", bufs=1))
        big = ctx.enter_context(tc.tile_pool(name="big", bufs=2))
        med = ctx.enter_context(tc.tile_pool(name="med", bufs=2))
        small = ctx.enter_context(tc.tile_pool(name="small", bufs=2))
        pp_small = ctx.enter_context(tc.tile_pool(name="pp_small", bufs=2, space="PSUM"))
        pp_vw = ctx.enter_context(tc.tile_pool(name="pp_vw", bufs=1, space="PSUM"))
        pp_lg = ctx.enter_context(tc.tile_pool(name="pp_lg", bufs=2, space="PSUM"))
        pp_rb = ctx.enter_context(tc.tile_pool(name="pp_rb", bufs=1, space="PSUM"))
        pp_o = ctx.enter_context(tc.tile_pool(name="pp_o", bufs=2, space="PSUM"))

        # ---- persistent weights / constants -------------------------------
        wvt_f = wpool.tile([128, 2 * C], F32)       # (hh, c) free layout
        nc.sync.dma_start(wvt_f[:], wvt_d.rearrange("(h p) c -> p h c", p=128))
        wvt_sb = wpool.tile([128, 2 * C], BF16)
        nc.vector.tensor_copy(wvt_sb[:], wvt_f[:])

        wk3_f = wpool.tile([128, 6], F32)
        nc.sync.dma_start(wk3_f[:], wk3_d.rearrange("(h p) k -> p h k", p=128))
        wk3_sb = wpool.tile([128, 6], BF16)
        nc.vector.tensor_copy(wk3_sb[:], wk3_f[:])

        aux_sb = wpool.tile([128, 6], F32)
        nc.sync.dma_start(aux_sb[:], aux_d)
        g1_sb = wpool.tile([128, GROUPS // 2], F32)
        nc.sync.dma_start(g1_sb[:], g1_d)
        g2_sb = wpool.tile([GROUPS // 2, 128], F32)
        nc.sync.dma_start(g2_sb[:], g2_d)

        ones_col = wpool.tile([128, 1], F32)
        nc.vector.memset(ones_col[:], 1.0)
        ones_row = wpool.tile([1, 128], F32)
        nc.vector.memset(ones_row[:], 1.0)
        # eps/2 as a per-partition bias AP (only 0.0/1.0 consts pre-registered)
        epsb = wpool.tile([GROUPS // 2, 1], F32)
        nc.vector.memset(epsb[:], EPS / 2)

        for s in range(BP):
            # ---- load x (pre-scaled by 1/sqrt(2)) -------------------------
            # xs layout: [128 part, (hh, spatial)]; channel = hh*128 + p
            xs = big.tile([128, 2 * S], F32, tag="xs")
            nc.sync.dma_start(xs[:], x_d[s].rearrange("(h p) w -> p h w", p=128))
            nc.vector.tensor_scalar_mul(xs[:], xs[:], R2)

            # ---- GroupNorm statistics -------------------------------------
            stats = small.tile([128, 4], F32, tag="stats")
            nc.vector.reduce_sum(
                stats[:, 0:2], xs[:].rearrange("p (h w) -> p h w", h=2),
                axis=mybir.AxisListType.X)
            sq = big.tile([128, 2 * S], BF16, tag="sq")
            for hh in range(2):
                nc.vector.scalar_tensor_tensor(
                    sq[:, hh * S:(hh + 1) * S],
                    xs[:, hh * S:(hh + 1) * S], 1.0, xs[:, hh * S:(hh + 1) * S],
                    mybir.AluOpType.mult, mybir.AluOpType.mult,
                    accum_out=stats[:, 2 + hh:3 + hh])
            # group-reduce: [16, 4] = g1^T @ stats  (groups g & g+16 per row)
            ps_g = pp_small.tile([GROUPS // 2, 4], F32, tag="ps_small")
            nc.tensor.matmul(ps_g[:], g1_sb[:], stats[:], start=True, stop=True)
            # per-group mean / inv-std
            gb4 = small.tile([GROUPS // 2, 4], F32, tag="gb4")
            inv_n = 1.0 / (CPG * S)
            nc.vector.tensor_scalar_mul(gb4[:, 0:2], ps_g[:, 0:2], inv_n)   # mean
            gtmp = small.tile([GROUPS // 2, 4], F32, tag="gtmp")
            nc.vector.tensor_scalar_mul(gtmp[:, 0:2], ps_g[:, 2:4], inv_n)  # E[x^2]
            nc.vector.tensor_mul(gtmp[:, 2:4], gb4[:, 0:2], gb4[:, 0:2])    # mean^2
            nc.vector.tensor_sub(gtmp[:, 0:2], gtmp[:, 0:2], gtmp[:, 2:4])  # var
            # sqrt(var + eps/2); eps halved because xs = x/sqrt(2)
            nc.scalar.activation(gtmp[:, 2:4], gtmp[:, 0:2],
                                 mybir.ActivationFunctionType.Sqrt, bias=epsb[:])
            nc.vector.reciprocal(gb4[:, 2:4], gtmp[:, 2:4])                 # inv-std
            # broadcast to channels: [128, 4] = g2^T @ gb4
            ps_cb = pp_small.tile([128, 4], F32, tag="ps_small")
            nc.tensor.matmul(ps_cb[:], g2_sb[:], gb4[:], start=True, stop=True)
            # per-channel a = gn_w * inv, b = gn_b - mean * a
            ab = small.tile([128, 4], F32, tag="ab")
            nc.vector.tensor_mul(ab[:, 0:2], aux_sb[:, 0:2], ps_cb[:, 2:4])
            abt = small.tile([128, 2], F32, tag="abt")
            nc.vector.tensor_mul(abt[:], ps_cb[:, 0:2], ab[:, 0:2])
            nc.vector.tensor_sub(ab[:, 2:4], aux_sb[:, 2:4], abt[:])
            # h2 = a*xs + b  (bf16, channel-major)
            h2 = med.tile([128, 2 * S], BF16, tag="h2")
            for hh in range(2):
                nc.vector.tensor_scalar(
                    h2[:, hh * S:(hh + 1) * S], xs[:, hh * S:(hh + 1) * S],
                    ab[:, hh:hh + 1], ab[:, 2 + hh:3 + hh],
                    mybir.AluOpType.mult, mybir.AluOpType.add)

            # ---- condition path: maxpool 4x4 + SiLU -> qori3 [3, S] -------
            # one DMA: partition = (chan, pooled_row), free = (raw_row a, col)
            cpool = med.tile([64, 512], F32, tag="cpool")
            nc.sync.dma_start(
                cpool[:].rearrange("p (a w) -> p a w", a=4),
                cm_d[s].rearrange("c (pr a) w -> (c pr) a w", a=4))
            # max over cols within 4-wide windows, then over the 4 raw rows
            prow = small.tile([64, 128], F32, tag="prow")
            nc.vector.reduce_max(
                prow[:], cpool[:].rearrange("p (a pc b) -> p a pc b", a=4, b=4),
                axis=mybir.AxisListType.X)
            pmax = small.tile([64, 32], F32, tag="pmax")
            nc.vector.reduce_max(
                pmax[:], prow[:].rearrange("p (a pc) -> p pc a", a=4),
                axis=mybir.AxisListType.X)
            qsig = small.tile([64, 32], F32, tag="qsig")
            nc.scalar.activation(qsig[:], pmax[:],
                                 mybir.ActivationFunctionType.Sigmoid)
            qsil = small.tile([64, 32], BF16, tag="qsil")
            nc.vector.tensor_mul(qsil[:], pmax[:], qsig[:])
            qori3 = small.tile([3, S], BF16, tag="qori3")
            nc.vector.memset(qori3[:], 1.0)   # row 2 stays the ones row
            nc.sync.dma_start(
                qori3[0:2, :].rearrange("c (pr pc) -> c pr pc", pr=32), qsil[:])

            # ---- kq3T [3, S] = (Wk3^T @ h2) -------------------------------
            kq3 = small.tile([3, S], BF16, tag="kq3")
            for ih in range(2):
                ps_kq = pp_small.tile([3, 512], F32, tag="ps_small")
                for hh in range(2):
                    nc.tensor.matmul(
                        ps_kq[:],
                        wk3_sb[:, hh * 3:(hh + 1) * 3],
                        h2[:, hh * S + ih * 512: hh * S + (ih + 1) * 512],
                        start=(hh == 0), stop=(hh == 1))
                nc.any.tensor_copy(kq3[:, ih * 512:(ih + 1) * 512], ps_kq[:])

            # ---- vw [S, C] = h2^T @ WvT  (j-major tiles) ------------------
            vw = med.tile([128, 8 * C], BF16, tag="vw")   # free = (jc, c)
            for jc in range(8):
                ps_vw = pp_vw.tile([128, C], F32, tag="ps_vw")
                for hh in range(2):
                    nc.tensor.matmul(
                        ps_vw[:],
                        h2[:, hh * S + jc * 128: hh * S + (jc + 1) * 128],
                        wvt_sb[:, hh * C:(hh + 1) * C],
                        start=(hh == 0), stop=(hh == 1))
                nc.any.tensor_copy(vw[:, jc * C:(jc + 1) * C], ps_vw[:])

            # ---- logits (rank-3) + exp ------------------------------------
            expT = big.tile([128, 8 * S], BF16, tag="expT")  # free = (jc, i)
            for jc in range(8):
                for ih in range(2):
                    ps_lg = pp_lg.tile([128, 512], F32, tag="ps_lg")
                    nc.tensor.matmul(
                        ps_lg[:],
                        kq3[:, jc * 128:(jc + 1) * 128],
                        qori3[:, ih * 512:(ih + 1) * 512],
                        start=True, stop=True)
                    nc.scalar.activation(
                        expT[:, jc * S + ih * 512: jc * S + (ih + 1) * 512],
                        ps_lg[:], mybir.ActivationFunctionType.Exp)

            # ---- softmax denominator -> broadcast reciprocal --------------
            acc = med.tile([128, S], F32, tag="acc")
            nc.gpsimd.tensor_add(acc[:], expT[:, 0:S], expT[:, S:2 * S])
            for jc in range(2, 8):
                nc.gpsimd.tensor_add(acc[:], acc[:], expT[:, jc * S:(jc + 1) * S])
            recip = small.tile([1, S], F32, tag="recip")
            for ih in range(2):
                ps_s = pp_small.tile([1, 512], F32, tag="ps_small")
                nc.tensor.matmul(ps_s[:], ones_col[:],
                                 acc[:, ih * 512:(ih + 1) * 512],
                                 start=True, stop=True)
                nc.vector.reciprocal(recip[:, ih * 512:(ih + 1) * 512], ps_s[:])
            recipB = med.tile([128, S], F32, tag="recipB")
            for ih in range(2):
                ps_rb = pp_rb.tile([128, 512], F32, tag="ps_rb")
                nc.tensor.matmul(ps_rb[:], ones_row[:],
                                 recip[:, ih * 512:(ih + 1) * 512],
                                 start=True, stop=True)
                nc.any.tensor_copy(recipB[:, ih * 512:(ih + 1) * 512], ps_rb[:])

            # ---- attn @ vw  -> outT [c, i], fused epilogue ----------------
            final = big.tile([128, 2 * S], F32, tag="final")
            for cc in range(2):
                for ih in range(2):
                    ps_o = pp_o.tile([128, 512], F32, tag="ps_o")
                    for jc in range(8):
                        nc.tensor.matmul(
                            ps_o[:],
                            vw[:, jc * C + cc * 128: jc * C + (cc + 1) * 128],
                            expT[:, jc * S + ih * 512: jc * S + (ih + 1) * 512],
                            start=(jc == 0), stop=(jc == 7))
                    t = med.tile([128, 512], F32, tag="ep_t")
                    sl = slice(cc * S + ih * 512, cc * S + (ih + 1) * 512)
                    nc.vector.tensor_mul(t[:], ps_o[:],
                                         recipB[:, ih * 512:(ih + 1) * 512])
                    nc.vector.scalar_tensor_tensor(
                        final[:, sl], t[:], aux_sb[:, 4 + cc:5 + cc], xs[:, sl],
                        mybir.AluOpType.add, mybir.AluOpType.add)

            nc.sync.dma_start(out_d[s].rearrange("(h p) w -> p h w", p=128),
                              final[:])

    nc.compile()   # bacc: register alloc, DCE, sync-wait fusion
    return nc


def _host_fold(gn_w, gn_b, fp1_w, fp1_b, fp2_w, fp2_b, out_w, out_b):
    scale2 = np.float32(1.0 / np.sqrt(C))          # (C**-0.25)^2
    fp1_wk, fp1_wv = fp1_w[:C], fp1_w[C:]
    fp1_bv = fp1_b[C:]
    wk3 = (fp1_wk.T @ np.concatenate([fp2_w, fp2_b[:, None]], 1)) * scale2  # [C,3]
    wvt = np.ascontiguousarray((fp1_wv.T @ out_w.T) * R2)                   # [C,C]
    bfin = (out_w @ fp1_bv + out_b) * R2                                    # [C]

    aux = np.empty((128, 6), np.float32)
    aux[:, 0:2] = gn_w.reshape(2, 128).T
    aux[:, 2:4] = gn_b.reshape(2, 128).T
    aux[:, 4:6] = bfin.reshape(2, 128).T

    # group indicator matrices (group g = channels 8g..8g+8; halves share rows)
    g1 = np.zeros((128, GROUPS // 2), np.float32)
    g1[np.arange(128), np.arange(128) // CPG] = 1.0
    g2 = np.ascontiguousarray(g1.T)
    return np.ascontiguousarray(wk3), wvt, aux, g1, g2


def kernel(x, cond_matrix, gn_w, gn_b, fp1_w, fp1_b, fp2_w, fp2_b, out_w, out_b):
    global LAST_RESULTS
    f = lambda a: np.ascontiguousarray(np.asarray(a, dtype=np.float32))
    x = f(x); cond_matrix = f(cond_matrix)
    gn_w, gn_b = f(gn_w), f(gn_b)
    fp1_w, fp1_b = f(fp1_w), f(fp1_b)
    fp2_w, fp2_b = f(fp2_w), f(fp2_b)
    out_w, out_b = f(out_w), f(out_b)

    wk3, wvt, aux, g1, g2 = _host_fold(gn_w, gn_b, fp1_w, fp1_b,
                                       fp2_w, fp2_b, out_w, out_b)

    key = "v1"
    if key not in _PROGRAM_CACHE:
        _PROGRAM_CACHE[key] = _build_program()
    nc = _PROGRAM_CACHE[key]

    xr = x.reshape(B, C, S)
    in_maps = []
    for c in range(N_CORES):
        in_maps.append({
            "x": xr[c * BP:(c + 1) * BP],
            "cond": cond_matrix[c * BP:(c + 1) * BP],
            "wvt": wvt, "wk3": wk3, "aux": aux, "g1": g1, "g2": g2,
        })

    res = bass_utils.run_bass_kernel_spmd(nc, in_maps, list(range(N_CORES)))
    LAST_RESULTS = res
    out = np.concatenate([res.results[c]["out"] for c in range(N_CORES)], axis=0)
    return np.ascontiguousarray(out.reshape(B, C, H, W).astype(np.float32))
